# revision 43
# baseline (speedup 1.0000x reference)
"""Multi-head self-attention Trainium2 kernel (Bass/Tile), batch-parallel
over 8 NeuronCores.

Problem (hardcoded): B=8, L=1024, D=1024, H=16, hd=64, f32 in/out.
  qkv = x @ w_qkv + b_qkv ; per-head scores = q k^T / 8 ; mask ; softmax ;
  out = (P v) heads-merged @ w_out + b_out.

Sharding: one batch element per core (data parallel); full weights on every
core. No collectives.

v3 dataflow (bf16 matmul path, f32 PSUM accumulation):
  - phase 1: qkvT[3D x L] = w_qkv^T @ x^T, 24 M-tiles x 8 K-chunks -> bf16.
  - phase 2 per head pair: V^T via PE transposes + vt tiles with a ones
    column (softmax denominator rides the attn.V matmul); score chunks for
    both sibling heads run row-packed (K=64, tile_position (0,0)/(64,0)).
    exp is split across engines so both score tiles free concurrently and
    neither engine bottlenecks: even head on ScalarE (table exp, additive
    mask bias), odd head on a custom 8-stage DVE op computing
    256*exp(s/8) ~ ((1+s/64)^2+1)^8 times a 0/1 mask multiplicand -- the
    constant 256 cancels in softmax (per-head consistency).
  - denominators ride the otr evacuation (ScalarE), are DMA-gathered into
    [4 x L] batch tiles, one reciprocal_approx_fast per 2 pairs, cast to
    f32r, and broadcast across partitions by a tiny sel-matmul; one DVE
    multiply normalizes -> ot_fin bf16 (deferred 2 pairs to keep PE busy).
  - phase 3: Y = ot^T @ w_out per Lq-tile + b_out broadcast add -> f32 DRAM
    (w_out/b_out prefetched during phase 2).
"""

import sys

import numpy as np

try:
    import concourse.bass as bass  # noqa: F401
except Exception:  # pragma: no cover - defensive path setup
    for p in ("/opt/trn_rl_repo", "/opt/pypackages"):
        if p not in sys.path:
            sys.path.insert(0, p)
    import concourse.bass as bass  # noqa: F401

from contextlib import ExitStack

import ml_dtypes

import concourse.dve_ops as dve_ops
import concourse.tile as tile
from concourse import bacc, mybir
from concourse.bass_utils import run_bass_kernel_spmd
from concourse.dve_spec import C0, C1, Spec, Src0, Src1, lower
from concourse.dve_uop import DveOpSpec
from concourse.masks import make_identity

F32 = mybir.dt.float32
F32R = mybir.dt.float32r
BF16 = mybir.dt.bfloat16

B, L, D = 8, 1024, 1024
H, HD = 16, 64
D3 = 3 * D
N_CORES = 8
PART = 128
NK = D // PART  # 8 contraction chunks
NM = D3 // PART  # 24 qkv output tiles
NLQ = L // PART  # 8 query tiles
NLK = L // PART  # 8 key tiles
MG = 2  # qkv M-tiles per PSUM group (paired for 4KB DMA lines)
VW = HD + 2  # 66: V chunk width in vt tiles (64 dims + ones col + pad)


def _ref_exp8_mask(in0, in1, c0, c1, c2):
    """256*exp(8*c0*in0) approx, masked: (((1+c0*in0)^2)+1)^8 * in1."""
    f = np.float32
    u = (in0.astype(f) * f(c0)).astype(f)
    v = (u + f(c1)).astype(f)
    x = (v * v + f(c1)).astype(f)
    y = (x * x).astype(f)
    z = (y * y).astype(f)
    q = (z * z).astype(f)
    return (q * in1.astype(f)).astype(f)


def _register_exp8_op():
    name = "EXP8_MASK_ANT"
    if name in dve_ops._SUB_OPCODE_FOR_NAME:
        return next(op for op in dve_ops.OPS if op.name == name)
    u = Src0 * C0
    v = u + C1
    x = v * v + C1
    y = x * x
    z = y * y
    q = z * z
    body = q * Src1
    spec = Spec(body=body, reference=_ref_exp8_mask)
    row = dve_ops._CUSTOM_DVE_ROW_BASE + len(dve_ops.OPS)
    assert row < 0x20
    dve_ops._SUB_OPCODE_FOR_NAME[name] = row
    shas = {}
    for ver in ("v3", "v4"):
        compiled = DveOpSpec(
            name=name, opcode=row, uops=lower(spec, ver=ver), rd1_en=True
        )
        shas[ver] = compiled.sha(ver)
    op = dve_ops.DveOp(name, spec, subdim=False, uops_sha=shas)
    dve_ops.OPS.append(op)
    dve_ops.CUSTOM_DVE_SPECS[name] = spec
    return op


EXP8_OP = _register_exp8_op()


def build_nc(debug=False):
    nc = bacc.Bacc("TRN2", target_bir_lowering=False, debug=False)

    xT = nc.dram_tensor("xT", (D, L), BF16, kind="ExternalInput").ap()
    # q/k thirds of w_qkv blocked on host, M-tiles paired for 4KB DMA lines:
    # wqk_blk[mp, p, i, k, c] = w_qkv[k*128 + p, (2*mp + i)*128 + c]
    wqk_blk = nc.dram_tensor(
        "wqk_blk", (2 * D // (2 * PART), PART, 2, NK, PART), BF16,
        kind="ExternalInput"
    ).ap()
    # v third kept [dmodel, vdim]-major, k-chunks paired for 4KB lines:
    # wv_blk[kp, p, i, c] = w_qkv[(2*kp + i)*128 + p, 2*D + c]
    wv_blk = nc.dram_tensor(
        "wv_blk", (NK // 2, PART, 2, D), BF16, kind="ExternalInput"
    ).ap()
    bqkv = nc.dram_tensor("bqkv", (D3,), F32, kind="ExternalInput").ap()
    wout = nc.dram_tensor("wout", (D, D), BF16, kind="ExternalInput").ap()
    bout = nc.dram_tensor("bout", (PART, D), BF16, kind="ExternalInput").ap()
    maskb = nc.dram_tensor("maskb", (L,), F32, kind="ExternalInput").ap()
    maskm = nc.dram_tensor("maskm", (L,), F32, kind="ExternalInput").ap()
    sel = nc.dram_tensor("sel", (2, PART), F32, kind="ExternalInput").ap()
    Y = nc.dram_tensor("Y", (L, D), F32, kind="ExternalOutput").ap()

    with tile.TileContext(nc) as tc, ExitStack() as ctx:
        singles = ctx.enter_context(tc.tile_pool(name="singles", bufs=1))

        sel_f = singles.tile([2, PART], F32)
        nc.sync.dma_start(sel_f[:], sel[:, :])
        sel_sb = singles.tile([2, PART], BF16)
        nc.vector.tensor_copy(sel_sb[:], sel_f[:])
        bqkv_sb = singles.tile([PART, NM], F32)
        nc.sync.dma_start(bqkv_sb[:], bqkv.rearrange("(c p) -> p c", p=PART))
        mb_sb = singles.tile([PART, NLK], F32)
        nc.sync.dma_start(mb_sb[:], maskb.rearrange("(c p) -> p c", p=PART))
        mm_sb = singles.tile([PART, NLK], F32)
        nc.sync.dma_start(mm_sb[:], maskm.rearrange("(c p) -> p c", p=PART))

        # warm the ScalarE exp table early (2.7us load hides under phase 1)
        warm_in = singles.tile([1, 8], F32)
        warm_out = singles.tile([1, 8], F32)
        nc.vector.memset(warm_in[:], 0.0)
        nc.scalar.activation(
            warm_out[:], warm_in[:], mybir.ActivationFunctionType.Exp
        )

        # ---- tiles that must survive across phases ----
        qkvT_pool = ctx.enter_context(tc.tile_pool(name="qkvT", bufs=1))
        qkvT = []
        for m in range(2 * NK):
            t = qkvT_pool.tile([PART, L], BF16, tag=f"qkvT{m}")
            qkvT.append(t)
        # V is produced token-major: vtok_all[:, t*1024 + d] = V[t*128 + p, d]
        vtok_all = qkvT_pool.tile([PART, NK * D], BF16, tag="vtok")

        ot_pool = ctx.enter_context(tc.tile_pool(name="otpool", bufs=1))
        ot_fin = []
        for j in range(NK):
            t = ot_pool.tile([PART, L], BF16, tag=f"ot{j}")
            ot_fin.append(t)

        wout_pool = ctx.enter_context(tc.tile_pool(name="woutp", bufs=1))

        # ================= phase 1: qkv projection =================
        with (
            tc.tile_pool(name="xt", bufs=1) as xt_pool,
            tc.tile_pool(name="wblk", bufs=2 * MG) as wblk_pool,
            tc.tile_pool(name="pq", bufs=4, space="PSUM") as pq_pool,
        ):
            xt = []
            for k in range(NK):
                t = xt_pool.tile([PART, L], BF16, tag=f"xt{k}")
                xt.append(t)

            def load_xt(k):
                nc.sync.dma_start(xt[k][:], xT[k * PART : (k + 1) * PART, :])

            load_xt(0)
            load_xt(1)
            xt_loaded = 2
            # part A: q/k thirds, dim-major (16 M-tiles, paired)
            for g in range(NK):
                ms = [g * MG + i for i in range(MG)]
                wt = wblk_pool.tile([PART, MG * NK * PART], BF16, tag="wblk")
                nc.sync.dma_start(
                    wt[:], wqk_blk[g].rearrange("p i k c -> p (i k c)")
                )
                pts = {}
                for m in ms:
                    pt = pq_pool.tile([PART, L], F32, tag="pq")
                    pts[m] = pt
                for k in range(NK):
                    while xt_loaded < min(NK, k + 2):
                        load_xt(xt_loaded)
                        xt_loaded += 1
                    for i, m in enumerate(ms):
                        off = i * NK * PART + k * PART
                        for nh in range(2):
                            nc.tensor.matmul(
                                pts[m][:, nh * 512 : (nh + 1) * 512],
                                wt[:, off : off + PART],
                                xt[k][:, nh * 512 : (nh + 1) * 512],
                                start=(k == 0),
                                stop=(k == NK - 1),
                            )
                for m in ms:
                    nc.scalar.activation(
                        qkvT[m][:],
                        pts[m][:],
                        mybir.ActivationFunctionType.Identity,
                        bias=bqkv_sb[:, m : m + 1],
                        scale=1.0,
                    )
            # part B: v third, token-major (x^T chunks stationary, w_v
            # streaming; v bias is folded into b_out on the host)
            wv_sb = []
            for kp in range(NK // 2):
                wvt = wblk_pool.tile([PART, 2 * D], BF16, tag="wblk")
                nc.sync.dma_start(
                    wvt[:], wv_blk[kp].rearrange("p i c -> p (i c)")
                )
                wv_sb.append(wvt)
            for t in range(NK):
                pt = pq_pool.tile([PART, L], F32, tag="pq")
                for k in range(NK):
                    for nh in range(2):
                        nc.tensor.matmul(
                            pt[:, nh * 512 : (nh + 1) * 512],
                            xt[k][:, t * PART : (t + 1) * PART],
                            wv_sb[k // 2][:, (k % 2) * D + nh * 512 :
                                          (k % 2) * D + (nh + 1) * 512],
                            start=(k == 0),
                            stop=(k == NK - 1),
                        )
                nc.scalar.activation(
                    vtok_all[:, t * D : (t + 1) * D],
                    pt[:],
                    mybir.ActivationFunctionType.Identity,
                    scale=1.0,
                )

        # ================= phase 2: attention per head pair =================
        with (
            tc.tile_pool(name="epool", bufs=12) as e_pool,
            tc.tile_pool(name="vtpool", bufs=4) as vt_pool,
            tc.tile_pool(name="otraw", bufs=5) as otraw_pool,
            tc.tile_pool(name="denp", bufs=1) as den_pool,
            tc.tile_pool(name="rcp", bufs=2) as rc_pool,
            tc.tile_pool(name="stp", bufs=6, space="PSUM") as st_pool,
            tc.tile_pool(name="pop", bufs=2, space="PSUM") as po_pool,
        ):
            # prefetch phase-3 weights while PE is busy here
            bout_sb = wout_pool.tile([PART, D], BF16, tag="bout")
            nc.sync.dma_start(bout_sb[:], bout[:, :])
            wo = []
            for k in range(NK):
                t = wout_pool.tile([PART, D], BF16, tag=f"wo{k}")
                nc.sync.dma_start(t[:], wout[k * PART : (k + 1) * PART, :])
                wo.append(t)

            # denominator batch tiles: batch b serves pairs 2b, 2b+1
            den_bf = []
            rc_fr = []
            for bn in range(4):
                dbf = den_pool.tile([4, L], BF16, tag=f"dbf{bn}")
                den_bf.append(dbf)
                rfr = den_pool.tile([4, L], BF16, tag=f"rfr{bn}")
                rc_fr.append(rfr)

            def emit_batch_recip(bn):
                dflt = den_pool.tile([4, L], F32, tag=f"dflt{bn % 2}")
                rflt = den_pool.tile([4, L], F32, tag=f"rflt{bn % 2}")
                nc.vector.tensor_copy(dflt[:], den_bf[bn][:])
                with nc.allow_low_precision(reason="approx denom reciprocal"):
                    nc.vector.reciprocal_approx_fast(rflt[:], dflt[:])
                nc.vector.tensor_copy(rc_fr[bn][:], rflt[:])

            def emit_vt_pair(j):
                """Gather the pair's V columns from the token-major vtok
                into one [128, 8*132] tile: block c = [64 even-head dims,
                ones, pad, 64 odd-head dims, ones, pad]. The vt ring has 4
                buffers and the copies never touch the ones columns, so
                only the first 4 allocations need the memset."""
                vt = vt_pool.tile([PART, NLK * 2 * VW], BF16, tag="vt")
                if j < 4:
                    nc.vector.memset(vt[:], 1.0)
                vsrc = vtok_all[:].rearrange("p (c d) -> p c d", d=D)
                vt4 = vt[:].rearrange("p (c s w) -> p c s w", s=2, w=VW)
                for side in range(2):
                    nc.vector.tensor_copy(
                        vt4[:, :, side, 0:HD],
                        vsrc[:, :, j * PART + side * HD : j * PART + side * HD + HD],
                    )
                return vt

            def emit_scores_exp(j, c, h):
                """One q-half score chunk for both sibling heads in two
                independent 1-bank PSUM units; the heads' matmuls are
                row-packed (adjacent issue, different row groups). exp split
                across ScalarE (even head, additive mask) and the DVE custom
                op (odd head, multiplicative mask)."""
                ns = slice(h * 512, (h + 1) * 512)
                st_e = st_pool.tile([PART, 512], F32, tag="st")
                st_o = st_pool.tile([PART, 512], F32, tag="st")
                for side, st in ((0, st_e), (1, st_o)):
                    ro = side * HD
                    nc.tensor.matmul(
                        st[:],
                        qkvT[NLQ + j][ro : ro + HD, c * PART : (c + 1) * PART],
                        qkvT[j][ro : ro + HD, ns],
                        start=True,
                        stop=True,
                        tile_position=(ro, 0),
                    )
                et_e = e_pool.tile([PART, 512], BF16, tag="e")
                nc.scalar.activation(
                    et_e[:],
                    st_e[:],
                    mybir.ActivationFunctionType.Exp,
                    bias=mb_sb[:, c : c + 1],
                    scale=1.0 / 8.0,
                )
                et_o = e_pool.tile([PART, 512], BF16, tag="e")
                nc.vector._custom_dve(
                    EXP8_OP,
                    out=et_o[:],
                    in0=st_o[:],
                    in1=mm_sb[:, c : c + 1].to_broadcast((PART, 512)),
                    s0=1.0 / 64.0,
                    s1=1.0,
                    imm2=0.0,
                )
                return [et_e, et_o]

            def flush_pending():
                pj, potr_e, potr_o = pending.pop(0)
                bn, row = pj // 2, (pj % 2) * 2
                rc2 = rc_pool.tile([2, L], BF16, tag="rc")
                nc.sync.dma_start(rc2[0:1, :], rc_fr[bn][row : row + 1, :])
                nc.sync.dma_start(rc2[1:2, :], rc_fr[bn][row + 1 : row + 2, :])
                for half in range(2):
                    ns = slice(half * 512, (half + 1) * 512)
                    rt = st_pool.tile([PART, 512], F32, tag="st")
                    nc.tensor.matmul(
                        rt[:], sel_sb[:], rc2[0:2, ns],
                        start=True, stop=True,
                    )
                    nc.vector.tensor_mul(
                        ot_fin[pj][0:HD, ns], potr_e[0:HD, ns], rt[0:HD, :]
                    )
                    nc.vector.tensor_mul(
                        ot_fin[pj][HD:PART, ns], potr_o[0:HD, ns], rt[HD:PART, :]
                    )

            pending = []
            # subpair sp = (pair j, q-half h); each is a self-contained
            # scores -> exp -> attn.V unit, so the ST ring holds 3 chunks
            # of slack and only two po banks are live at a time
            NSP = 2 * NK
            vts = {0: emit_vt_pair(0)}
            otr_pair = {}
            ets_q = {0: [emit_scores_exp(0, 0, 0)]}

            for sp in range(NSP):
                j, h = sp // 2, sp % 2
                ns = slice(h * 512, (h + 1) * 512)
                vt = vts[j]
                po_e = po_pool.tile([HD + 1, 512], F32, tag="po")
                po_o = po_pool.tile([HD + 1, 512], F32, tag="po")
                po = [po_e, po_o]
                if h == 0:
                    otr_e = otraw_pool.tile([HD + 1, L], BF16, tag="otre")
                    otr_o = otraw_pool.tile([HD + 1, L], BF16, tag="otro")
                    otr_pair[j] = (otr_e, otr_o)
                otrs = otr_pair[j]

                for c in range(NLK):
                    # scores+exp for the NEXT chunk (this or next subpair)
                    if c < NLK - 1:
                        ets_q[sp].append(emit_scores_exp(j, c + 1, h))
                    else:
                        if h == 1 and j + 1 < NK:
                            vts[j + 1] = emit_vt_pair(j + 1)
                        if sp + 1 < NSP:
                            nj, nh2 = (sp + 1) // 2, (sp + 1) % 2
                            ets_q[sp + 1] = [emit_scores_exp(nj, 0, nh2)]
                    if h == 0 and c == 4 and pending and pending[0][0] <= j - 2:
                        flush_pending()
                    et_pair = ets_q[sp][c]
                    for side in range(2):
                        voff = c * 2 * VW + side * VW
                        nc.tensor.matmul(
                            po[side][:],
                            vt[:, voff : voff + HD + 1],
                            et_pair[side][:],
                            start=(c == 0),
                            stop=(c == NLK - 1),
                        )
                del ets_q[sp]

                # evacuate unnormalized outputs + denominator row, split
                # across ScalarE (even head) and DVE (odd head) so po banks
                # free fast without serializing either engine's queue
                nc.scalar.activation(
                    otrs[0][0 : HD + 1, ns],
                    po_e[0 : HD + 1, :],
                    mybir.ActivationFunctionType.Identity,
                    scale=1.0,
                )
                nc.vector.tensor_copy(otrs[1][0 : HD + 1, ns], po_o[0 : HD + 1, :])
                if h == 1:
                    vts.pop(j)
                    for side in range(2):
                        nc.sync.dma_start(
                            den_bf[j // 2][
                                (j % 2) * 2 + side : (j % 2) * 2 + side + 1, :
                            ],
                            otrs[side][HD : HD + 1, :],
                        )
                    pending.append((j, otrs[0], otrs[1]))
                    del otr_pair[j]
                    if j % 2 == 1:
                        emit_batch_recip(j // 2)

            while pending:
                flush_pending()

            # ============ phase 3: output projection (same pools) ============
            # pf units come from the po ring, so the first lq tiles start
            # while the last pair's normalization drains
            with tc.tile_pool(name="fsb", bufs=4) as f_pool:
                for lq in range(NLQ):
                    for nh in range(2):
                        ns = slice(nh * 512, (nh + 1) * 512)
                        pf = po_pool.tile([PART, 512], F32, tag="po")
                        for k in range(NK):
                            nc.tensor.matmul(
                                pf[:],
                                ot_fin[k][:, lq * PART : (lq + 1) * PART],
                                wo[k][:, ns],
                                start=(k == 0),
                                stop=(k == NK - 1),
                            )
                        fs = f_pool.tile([PART, 512], F32, tag="fsb")
                        nc.vector.tensor_add(fs[:], pf[:], bout_sb[:, ns])
                        nc.sync.dma_start(Y[lq * PART : (lq + 1) * PART, ns], fs[:])

    nc.compile()
    return nc


_NC_CACHE = None


def _get_nc():
    global _NC_CACHE
    if _NC_CACHE is None:
        _NC_CACHE = build_nc()
    return _NC_CACHE


def make_in_maps(x, attn_mask, w_qkv, b_qkv, w_out, b_out):
    """Host-side sharding + layout prep -> per-core input maps."""
    bf16 = ml_dtypes.bfloat16
    x = np.asarray(x, dtype=np.float32)
    attn_mask = np.asarray(attn_mask)
    w_qkv = np.asarray(w_qkv, dtype=np.float32)
    b_qkv = np.ascontiguousarray(np.asarray(b_qkv, dtype=np.float32))
    w_out = np.ascontiguousarray(np.asarray(w_out, dtype=np.float32).astype(bf16))
    b_out = np.asarray(b_out, dtype=np.float32).astype(bf16)

    # wqk_blk[mp, p, i, k, c] = w_qkv[k*128 + p, (2*mp + i)*128 + c]
    wqk = np.ascontiguousarray(
        w_qkv[:, : 2 * D]
        .reshape(NK, PART, D // PART, 2, PART)
        .transpose(2, 1, 3, 0, 4)
        .astype(bf16)
    )
    # wv_blk[kp, p, i, c] = w_qkv[(2*kp + i)*128 + p, 2*D + c]
    wv = np.ascontiguousarray(
        w_qkv[:, 2 * D :].reshape(NK // 2, 2, PART, D).transpose(0, 2, 1, 3).astype(bf16)
    )
    # v bias is exactly additive post-softmax, so fold it into b_out
    b_out_eff = (
        np.asarray(b_out, dtype=np.float64)
        + np.asarray(b_qkv, dtype=np.float64)[2 * D :] @ np.asarray(w_out, dtype=np.float64)
    ).astype(np.float32).astype(bf16)
    mvalid = attn_mask.astype(bool)
    maskbias = np.where(mvalid, 0.0, -10000.0).astype(np.float32)
    maskmul = mvalid.astype(np.float32)

    sel_host = np.zeros((2, PART), dtype=np.float32)
    sel_host[0, 0:HD] = 1.0
    sel_host[1, HD:PART] = 1.0
    in_maps = []
    for b in range(B):
        in_maps.append(
            {
                "xT": np.ascontiguousarray(x[b].T.astype(bf16)),
                "wqk_blk": wqk,
                "wv_blk": wv,
                "bqkv": b_qkv,
                "wout": w_out,
                "bout": np.ascontiguousarray(np.broadcast_to(b_out_eff, (PART, D))),
                "maskb": np.ascontiguousarray(maskbias[b]),
                "maskm": np.ascontiguousarray(maskmul[b]),
                "sel": sel_host,
            }
        )
    return in_maps


def kernel(x, attn_mask, w_qkv, b_qkv, w_out, b_out):
    in_maps = make_in_maps(x, attn_mask, w_qkv, b_qkv, w_out, b_out)
    nc = _get_nc()
    res = run_bass_kernel_spmd(nc, in_maps, core_ids=list(range(N_CORES)))
    return np.stack([res.results[b]["Y"] for b in range(B)], axis=0)


if __name__ == "__main__":
    rng = np.random.default_rng(0)
    inputs = {
        "x": rng.standard_normal((B, L, D), dtype=np.float32),
        "attn_mask": np.ones((B, L), dtype=bool),
        "w_qkv": ((rng.random((D, D3), dtype=np.float32) - 0.5) / 16.0),
        "b_qkv": np.zeros((D3,), dtype=np.float32),
        "w_out": ((rng.random((D, D), dtype=np.float32) - 0.5) / 16.0),
        "b_out": np.zeros((D,), dtype=np.float32),
    }
    y = kernel(**inputs)
    print(y.shape, y.dtype)


# revision 44
# speedup vs baseline: 1.0735x; 1.0735x over previous
"""Multi-head self-attention Trainium2 kernel (Bass/Tile), batch-parallel
over 8 NeuronCores.

Problem (hardcoded): B=8, L=1024, D=1024, H=16, hd=64, f32 in/out.
  qkv = x @ w_qkv + b_qkv ; per-head scores = q k^T / 8 ; mask ; softmax ;
  out = (P v) heads-merged @ w_out + b_out.

Sharding: one batch element per core (data parallel); full weights on every
core. No collectives.

v3 dataflow (bf16 matmul path, f32 PSUM accumulation):
  - phase 1: qkvT[3D x L] = w_qkv^T @ x^T, 24 M-tiles x 8 K-chunks -> bf16.
  - phase 2 per head pair: V^T via PE transposes + vt tiles with a ones
    column (softmax denominator rides the attn.V matmul); score chunks for
    both sibling heads run row-packed (K=64, tile_position (0,0)/(64,0)).
    exp is split across engines so both score tiles free concurrently and
    neither engine bottlenecks: even head on ScalarE (table exp, additive
    mask bias), odd head on a custom 8-stage DVE op computing
    256*exp(s/8) ~ ((1+s/64)^2+1)^8 times a 0/1 mask multiplicand -- the
    constant 256 cancels in softmax (per-head consistency).
  - denominators ride the otr evacuation (ScalarE), are DMA-gathered into
    [4 x L] batch tiles, one reciprocal_approx_fast per 2 pairs, cast to
    f32r, and broadcast across partitions by a tiny sel-matmul; one DVE
    multiply normalizes -> ot_fin bf16 (deferred 2 pairs to keep PE busy).
  - phase 3: Y = ot^T @ w_out per Lq-tile + b_out broadcast add -> f32 DRAM
    (w_out/b_out prefetched during phase 2).
"""

import sys

import numpy as np

try:
    import concourse.bass as bass  # noqa: F401
except Exception:  # pragma: no cover - defensive path setup
    for p in ("/opt/trn_rl_repo", "/opt/pypackages"):
        if p not in sys.path:
            sys.path.insert(0, p)
    import concourse.bass as bass  # noqa: F401

from contextlib import ExitStack

import ml_dtypes

import concourse.dve_ops as dve_ops
import concourse.tile as tile
from concourse import bacc, mybir
from concourse.bass_utils import run_bass_kernel_spmd
from concourse.dve_spec import C0, C1, Spec, Src0, Src1, lower
from concourse.dve_uop import DveOpSpec
from concourse.masks import make_identity

F32 = mybir.dt.float32
F32R = mybir.dt.float32r
BF16 = mybir.dt.bfloat16

B, L, D = 8, 1024, 1024
H, HD = 16, 64
D3 = 3 * D
N_CORES = 8
PART = 128
NK = D // PART  # 8 contraction chunks
NM = D3 // PART  # 24 qkv output tiles
NLQ = L // PART  # 8 query tiles
NLK = L // PART  # 8 key tiles
MG = 2  # qkv M-tiles per PSUM group (paired for 4KB DMA lines)
VW = HD + 2  # 66: V chunk width in vt tiles (64 dims + ones col + pad)


def _ref_exp8_mask(in0, in1, c0, c1, c2):
    """256*exp(8*c0*in0) approx, masked: (((1+c0*in0)^2)+1)^8 * in1."""
    f = np.float32
    u = (in0.astype(f) * f(c0)).astype(f)
    v = (u + f(c1)).astype(f)
    x = (v * v + f(c1)).astype(f)
    y = (x * x).astype(f)
    z = (y * y).astype(f)
    q = (z * z).astype(f)
    return (q * in1.astype(f)).astype(f)


def _register_exp8_op():
    name = "EXP8_MASK_ANT"
    if name in dve_ops._SUB_OPCODE_FOR_NAME:
        return next(op for op in dve_ops.OPS if op.name == name)
    u = Src0 * C0
    v = u + C1
    x = v * v + C1
    y = x * x
    z = y * y
    q = z * z
    body = q * Src1
    spec = Spec(body=body, reference=_ref_exp8_mask)
    row = dve_ops._CUSTOM_DVE_ROW_BASE + len(dve_ops.OPS)
    assert row < 0x20
    dve_ops._SUB_OPCODE_FOR_NAME[name] = row
    shas = {}
    for ver in ("v3", "v4"):
        compiled = DveOpSpec(
            name=name, opcode=row, uops=lower(spec, ver=ver), rd1_en=True
        )
        shas[ver] = compiled.sha(ver)
    op = dve_ops.DveOp(name, spec, subdim=False, uops_sha=shas)
    dve_ops.OPS.append(op)
    dve_ops.CUSTOM_DVE_SPECS[name] = spec
    return op


EXP8_OP = _register_exp8_op()


def build_nc(debug=False):
    nc = bacc.Bacc("TRN2", target_bir_lowering=False, debug=False)

    xT = nc.dram_tensor("xT", (D, L), BF16, kind="ExternalInput").ap()
    # q/k thirds of w_qkv blocked on host, M-tiles paired for 4KB DMA lines:
    # wqk_blk[mp, p, i, k, c] = w_qkv[k*128 + p, (2*mp + i)*128 + c]
    wqk_blk = nc.dram_tensor(
        "wqk_blk", (2 * D // (2 * PART), PART, 2, NK, PART), BF16,
        kind="ExternalInput"
    ).ap()
    # v third kept [dmodel, vdim]-major, k-chunks paired for 4KB lines:
    # wv_blk[kp, p, i, c] = w_qkv[(2*kp + i)*128 + p, 2*D + c]
    wv_blk = nc.dram_tensor(
        "wv_blk", (NK // 2, PART, 2, D), BF16, kind="ExternalInput"
    ).ap()
    bqkv = nc.dram_tensor("bqkv", (D3,), F32, kind="ExternalInput").ap()
    wout = nc.dram_tensor("wout", (D, D), BF16, kind="ExternalInput").ap()
    bout = nc.dram_tensor("bout", (PART, D), BF16, kind="ExternalInput").ap()
    maskb = nc.dram_tensor("maskb", (L,), F32, kind="ExternalInput").ap()
    maskm = nc.dram_tensor("maskm", (L,), F32, kind="ExternalInput").ap()
    sel = nc.dram_tensor("sel", (2, PART), F32, kind="ExternalInput").ap()
    Y = nc.dram_tensor("Y", (L, D), F32, kind="ExternalOutput").ap()

    with tile.TileContext(nc) as tc, ExitStack() as ctx:
        singles = ctx.enter_context(tc.tile_pool(name="singles", bufs=1))

        sel_f = singles.tile([2, PART], F32)
        nc.sync.dma_start(sel_f[:], sel[:, :])
        sel_sb = singles.tile([2, PART], BF16)
        nc.vector.tensor_copy(sel_sb[:], sel_f[:])
        bqkv_sb = singles.tile([PART, NM], F32)
        nc.sync.dma_start(bqkv_sb[:], bqkv.rearrange("(c p) -> p c", p=PART))
        mb_sb = singles.tile([PART, NLK], F32)
        nc.sync.dma_start(mb_sb[:], maskb.rearrange("(c p) -> p c", p=PART))
        mm_sb = singles.tile([PART, NLK], F32)
        nc.sync.dma_start(mm_sb[:], maskm.rearrange("(c p) -> p c", p=PART))

        # warm the ScalarE exp table early (2.7us load hides under phase 1)
        warm_in = singles.tile([1, 8], F32)
        warm_out = singles.tile([1, 8], F32)
        nc.vector.memset(warm_in[:], 0.0)
        nc.scalar.activation(
            warm_out[:], warm_in[:], mybir.ActivationFunctionType.Exp
        )

        # ---- tiles that must survive across phases ----
        qkvT_pool = ctx.enter_context(tc.tile_pool(name="qkvT", bufs=1))
        qkvT = []
        for m in range(2 * NK):
            t = qkvT_pool.tile([PART, L], BF16, tag=f"qkvT{m}")
            qkvT.append(t)
        # V is produced token-major: vtok_all[:, t*1024 + d] = V[t*128 + p, d]
        vtok_all = qkvT_pool.tile([PART, NK * D], BF16, tag="vtok")

        ot_pool = ctx.enter_context(tc.tile_pool(name="otpool", bufs=1))
        ot_fin = []
        for j in range(NK):
            t = ot_pool.tile([PART, L], BF16, tag=f"ot{j}")
            ot_fin.append(t)

        wout_pool = ctx.enter_context(tc.tile_pool(name="woutp", bufs=1))

        # ================= phase 1: qkv projection =================
        with (
            tc.tile_pool(name="xt", bufs=1) as xt_pool,
            tc.tile_pool(name="wblk", bufs=2 * MG) as wblk_pool,
            tc.tile_pool(name="pq", bufs=4, space="PSUM") as pq_pool,
        ):
            xt = []
            for k in range(NK):
                t = xt_pool.tile([PART, L], BF16, tag=f"xt{k}")
                xt.append(t)

            def load_xt(k):
                nc.sync.dma_start(xt[k][:], xT[k * PART : (k + 1) * PART, :])

            load_xt(0)
            load_xt(1)
            xt_loaded = 2
            # part A: q/k thirds, dim-major (16 M-tiles, paired)
            for g in range(NK):
                ms = [g * MG + i for i in range(MG)]
                wt = wblk_pool.tile([PART, MG * NK * PART], BF16, tag="wblk")
                nc.sync.dma_start(
                    wt[:], wqk_blk[g].rearrange("p i k c -> p (i k c)")
                )
                pts = {}
                for m in ms:
                    pt = pq_pool.tile([PART, L], F32, tag="pq")
                    pts[m] = pt
                for k in range(NK):
                    while xt_loaded < min(NK, k + 2):
                        load_xt(xt_loaded)
                        xt_loaded += 1
                    for i, m in enumerate(ms):
                        off = i * NK * PART + k * PART
                        for nh in range(2):
                            nc.tensor.matmul(
                                pts[m][:, nh * 512 : (nh + 1) * 512],
                                wt[:, off : off + PART],
                                xt[k][:, nh * 512 : (nh + 1) * 512],
                                start=(k == 0),
                                stop=(k == NK - 1),
                            )
                for m in ms:
                    nc.scalar.activation(
                        qkvT[m][:],
                        pts[m][:],
                        mybir.ActivationFunctionType.Identity,
                        bias=bqkv_sb[:, m : m + 1],
                        scale=1.0,
                    )
            # part B: v third, token-major (x^T chunks stationary, w_v
            # streaming; v bias is folded into b_out on the host)
            wv_sb = []
            for kp in range(NK // 2):
                wvt = wblk_pool.tile([PART, 2 * D], BF16, tag="wblk")
                nc.sync.dma_start(
                    wvt[:], wv_blk[kp].rearrange("p i c -> p (i c)")
                )
                wv_sb.append(wvt)
            for t in range(NK):
                pt = pq_pool.tile([PART, L], F32, tag="pq")
                for k in range(NK):
                    for nh in range(2):
                        nc.tensor.matmul(
                            pt[:, nh * 512 : (nh + 1) * 512],
                            xt[k][:, t * PART : (t + 1) * PART],
                            wv_sb[k // 2][:, (k % 2) * D + nh * 512 :
                                          (k % 2) * D + (nh + 1) * 512],
                            start=(k == 0),
                            stop=(k == NK - 1),
                        )
                nc.scalar.activation(
                    vtok_all[:, t * D : (t + 1) * D],
                    pt[:],
                    mybir.ActivationFunctionType.Identity,
                    scale=1.0,
                )

        # ================= phase 2: attention per head pair =================
        with (
            tc.tile_pool(name="epool", bufs=12) as e_pool,
            tc.tile_pool(name="vtpool", bufs=4) as vt_pool,
            tc.tile_pool(name="otraw", bufs=5) as otraw_pool,
            tc.tile_pool(name="denp", bufs=1) as den_pool,
            tc.tile_pool(name="rcp", bufs=2) as rc_pool,
            tc.tile_pool(name="stp", bufs=4, space="PSUM") as st_pool,
            tc.tile_pool(name="pop", bufs=4, space="PSUM") as po_pool,
        ):
            # prefetch phase-3 weights while PE is busy here
            bout_sb = wout_pool.tile([PART, D], BF16, tag="bout")
            nc.sync.dma_start(bout_sb[:], bout[:, :])
            wo = []
            for k in range(NK):
                t = wout_pool.tile([PART, D], BF16, tag=f"wo{k}")
                nc.sync.dma_start(t[:], wout[k * PART : (k + 1) * PART, :])
                wo.append(t)

            # denominator batch tiles: batch b serves pairs 2b, 2b+1
            den_bf = []
            rc_fr = []
            for bn in range(4):
                dbf = den_pool.tile([4, L], BF16, tag=f"dbf{bn}")
                den_bf.append(dbf)
                rfr = den_pool.tile([4, L], BF16, tag=f"rfr{bn}")
                rc_fr.append(rfr)

            def emit_batch_recip(bn):
                dflt = den_pool.tile([4, L], F32, tag=f"dflt{bn % 2}")
                rflt = den_pool.tile([4, L], F32, tag=f"rflt{bn % 2}")
                nc.vector.tensor_copy(dflt[:], den_bf[bn][:])
                with nc.allow_low_precision(reason="approx denom reciprocal"):
                    nc.vector.reciprocal_approx_fast(rflt[:], dflt[:])
                nc.vector.tensor_copy(rc_fr[bn][:], rflt[:])

            def emit_vt_pair(j):
                """Gather the pair's V columns from the token-major vtok
                into one [128, 8*132] tile: block c = [64 even-head dims,
                ones, pad, 64 odd-head dims, ones, pad]. The vt ring has 4
                buffers and the copies never touch the ones columns, so
                only the first 4 allocations need the memset."""
                vt = vt_pool.tile([PART, NLK * 2 * VW], BF16, tag="vt")
                if j < 4:
                    nc.vector.memset(vt[:], 1.0)
                vsrc = vtok_all[:].rearrange("p (c d) -> p c d", d=D)
                vt4 = vt[:].rearrange("p (c s w) -> p c s w", s=2, w=VW)
                for side in range(2):
                    nc.vector.tensor_copy(
                        vt4[:, :, side, 0:HD],
                        vsrc[:, :, j * PART + side * HD : j * PART + side * HD + HD],
                    )
                return vt

            def emit_scores_exp(j, c):
                """Score chunks for both sibling heads in four independent
                1-bank PSUM units; sibling heads' matmuls are row-packed
                (adjacent issue, different row groups). exp split across
                ScalarE (even head, additive mask) and the DVE custom op
                (odd head, multiplicative mask)."""
                st_e0 = st_pool.tile([PART, 512], F32, tag="st")
                st_o0 = st_pool.tile([PART, 512], F32, tag="st")
                st_e1 = st_pool.tile([PART, 512], F32, tag="st")
                st_o1 = st_pool.tile([PART, 512], F32, tag="st")
                sts = [[st_e0, st_e1], [st_o0, st_o1]]
                ets = [[None, None], [None, None]]
                for nh in range(2):
                    ns = slice(nh * 512, (nh + 1) * 512)
                    for side in range(2):
                        ro = side * HD
                        nc.tensor.matmul(
                            sts[side][nh][:],
                            qkvT[NLQ + j][ro : ro + HD, c * PART : (c + 1) * PART],
                            qkvT[j][ro : ro + HD, ns],
                            start=True,
                            stop=True,
                            tile_position=(ro, 0),
                        )
                for nh in range(2):
                    et_e = e_pool.tile([PART, 512], BF16, tag="e")
                    nc.scalar.activation(
                        et_e[:],
                        sts[0][nh][:],
                        mybir.ActivationFunctionType.Exp,
                        bias=mb_sb[:, c : c + 1],
                        scale=1.0 / 8.0,
                    )
                    ets[0][nh] = et_e
                    et_o = e_pool.tile([PART, 512], BF16, tag="e")
                    nc.vector._custom_dve(
                        EXP8_OP,
                        out=et_o[:],
                        in0=sts[1][nh][:],
                        in1=mm_sb[:, c : c + 1].to_broadcast((PART, 512)),
                        s0=1.0 / 64.0,
                        s1=1.0,
                        imm2=0.0,
                    )
                    ets[1][nh] = et_o
                return ets

            def flush_pending():
                pj, potr_e, potr_o = pending.pop(0)
                bn, row = pj // 2, (pj % 2) * 2
                rc2 = rc_pool.tile([2, L], BF16, tag="rc")
                nc.sync.dma_start(rc2[0:1, :], rc_fr[bn][row : row + 1, :])
                nc.sync.dma_start(rc2[1:2, :], rc_fr[bn][row + 1 : row + 2, :])
                for half in range(2):
                    ns = slice(half * 512, (half + 1) * 512)
                    rt = st_pool.tile([PART, 512], F32, tag="st")
                    nc.tensor.matmul(
                        rt[:], sel_sb[:], rc2[0:2, ns],
                        start=True, stop=True,
                    )
                    nc.vector.tensor_mul(
                        ot_fin[pj][0:HD, ns], potr_e[0:HD, ns], rt[0:HD, :]
                    )
                    nc.vector.tensor_mul(
                        ot_fin[pj][HD:PART, ns], potr_o[0:HD, ns], rt[HD:PART, :]
                    )

            pending = []
            # prologue: pair 0 V gather + first score chunks
            vts = {0: emit_vt_pair(0)}
            ets_q = {0: [emit_scores_exp(0, 0)]}

            for j in range(NK):  # head pairs
                vt = vts.pop(j)
                po_e0 = po_pool.tile([HD + 1, 512], F32, tag="po")
                po_e1 = po_pool.tile([HD + 1, 512], F32, tag="po")
                po_o0 = po_pool.tile([HD + 1, 512], F32, tag="po")
                po_o1 = po_pool.tile([HD + 1, 512], F32, tag="po")
                po = [[po_e0, po_e1], [po_o0, po_o1]]
                otr_e = otraw_pool.tile([HD + 1, L], BF16, tag="otre")
                otr_o = otraw_pool.tile([HD + 1, L], BF16, tag="otro")
                otrs = [otr_e, otr_o]

                for c in range(NLK):
                    # scores+exp for the NEXT chunk (this pair or next pair)
                    if c < NLK - 1:
                        ets_q[j].append(emit_scores_exp(j, c + 1))
                    else:
                        if j + 1 < NK:
                            vts[j + 1] = emit_vt_pair(j + 1)
                            ets_q[j + 1] = [emit_scores_exp(j + 1, 0)]
                    if c == 4 and pending and pending[0][0] <= j - 2:
                        flush_pending()
                    et_pair = ets_q[j][c]
                    for side in range(2):
                        voff = c * 2 * VW + side * VW
                        for half in range(2):
                            nc.tensor.matmul(
                                po[side][half][:],
                                vt[:, voff : voff + HD + 1],
                                et_pair[side][half][:],
                                start=(c == 0),
                                stop=(c == NLK - 1),
                            )
                del ets_q[j]

                # evacuate unnormalized outputs + denominator row, split
                # across ScalarE (even head) and DVE (odd head) so po banks
                # free fast without serializing either engine's queue
                for half in range(2):
                    ns = slice(half * 512, (half + 1) * 512)
                    nc.scalar.activation(
                        otrs[0][0 : HD + 1, ns],
                        po[0][half][0 : HD + 1, :],
                        mybir.ActivationFunctionType.Identity,
                        scale=1.0,
                    )
                    nc.vector.tensor_copy(
                        otrs[1][0 : HD + 1, ns], po[1][half][0 : HD + 1, :]
                    )
                for side in range(2):
                    nc.sync.dma_start(
                        den_bf[j // 2][(j % 2) * 2 + side : (j % 2) * 2 + side + 1, :],
                        otrs[side][HD : HD + 1, :],
                    )
                pending.append((j, otr_e, otr_o))
                if j % 2 == 1:
                    emit_batch_recip(j // 2)

            while pending:
                flush_pending()

            # ============ phase 3: output projection (same pools) ============
            # pf units come from the po ring, so the first lq tiles start
            # while the last pair's normalization drains
            with tc.tile_pool(name="fsb", bufs=4) as f_pool:
                for lq in range(NLQ):
                    for nh in range(2):
                        ns = slice(nh * 512, (nh + 1) * 512)
                        pf = po_pool.tile([PART, 512], F32, tag="po")
                        for k in range(NK):
                            nc.tensor.matmul(
                                pf[:],
                                ot_fin[k][:, lq * PART : (lq + 1) * PART],
                                wo[k][:, ns],
                                start=(k == 0),
                                stop=(k == NK - 1),
                            )
                        fs = f_pool.tile([PART, 512], F32, tag="fsb")
                        nc.vector.tensor_add(fs[:], pf[:], bout_sb[:, ns])
                        nc.sync.dma_start(Y[lq * PART : (lq + 1) * PART, ns], fs[:])

    nc.compile()
    return nc


_NC_CACHE = None


def _get_nc():
    global _NC_CACHE
    if _NC_CACHE is None:
        _NC_CACHE = build_nc()
    return _NC_CACHE


def make_in_maps(x, attn_mask, w_qkv, b_qkv, w_out, b_out):
    """Host-side sharding + layout prep -> per-core input maps."""
    bf16 = ml_dtypes.bfloat16
    x = np.asarray(x, dtype=np.float32)
    attn_mask = np.asarray(attn_mask)
    w_qkv = np.asarray(w_qkv, dtype=np.float32)
    b_qkv = np.ascontiguousarray(np.asarray(b_qkv, dtype=np.float32))
    w_out = np.ascontiguousarray(np.asarray(w_out, dtype=np.float32).astype(bf16))
    b_out = np.asarray(b_out, dtype=np.float32).astype(bf16)

    # wqk_blk[mp, p, i, k, c] = w_qkv[k*128 + p, (2*mp + i)*128 + c]
    wqk = np.ascontiguousarray(
        w_qkv[:, : 2 * D]
        .reshape(NK, PART, D // PART, 2, PART)
        .transpose(2, 1, 3, 0, 4)
        .astype(bf16)
    )
    # wv_blk[kp, p, i, c] = w_qkv[(2*kp + i)*128 + p, 2*D + c]
    wv = np.ascontiguousarray(
        w_qkv[:, 2 * D :].reshape(NK // 2, 2, PART, D).transpose(0, 2, 1, 3).astype(bf16)
    )
    # v bias is exactly additive post-softmax, so fold it into b_out
    b_out_eff = (
        np.asarray(b_out, dtype=np.float64)
        + np.asarray(b_qkv, dtype=np.float64)[2 * D :] @ np.asarray(w_out, dtype=np.float64)
    ).astype(np.float32).astype(bf16)
    mvalid = attn_mask.astype(bool)
    maskbias = np.where(mvalid, 0.0, -10000.0).astype(np.float32)
    maskmul = mvalid.astype(np.float32)

    sel_host = np.zeros((2, PART), dtype=np.float32)
    sel_host[0, 0:HD] = 1.0
    sel_host[1, HD:PART] = 1.0
    in_maps = []
    for b in range(B):
        in_maps.append(
            {
                "xT": np.ascontiguousarray(x[b].T.astype(bf16)),
                "wqk_blk": wqk,
                "wv_blk": wv,
                "bqkv": b_qkv,
                "wout": w_out,
                "bout": np.ascontiguousarray(np.broadcast_to(b_out_eff, (PART, D))),
                "maskb": np.ascontiguousarray(maskbias[b]),
                "maskm": np.ascontiguousarray(maskmul[b]),
                "sel": sel_host,
            }
        )
    return in_maps


def kernel(x, attn_mask, w_qkv, b_qkv, w_out, b_out):
    in_maps = make_in_maps(x, attn_mask, w_qkv, b_qkv, w_out, b_out)
    nc = _get_nc()
    res = run_bass_kernel_spmd(nc, in_maps, core_ids=list(range(N_CORES)))
    return np.stack([res.results[b]["Y"] for b in range(B)], axis=0)


if __name__ == "__main__":
    rng = np.random.default_rng(0)
    inputs = {
        "x": rng.standard_normal((B, L, D), dtype=np.float32),
        "attn_mask": np.ones((B, L), dtype=bool),
        "w_qkv": ((rng.random((D, D3), dtype=np.float32) - 0.5) / 16.0),
        "b_qkv": np.zeros((D3,), dtype=np.float32),
        "w_out": ((rng.random((D, D), dtype=np.float32) - 0.5) / 16.0),
        "b_out": np.zeros((D,), dtype=np.float32),
    }
    y = kernel(**inputs)
    print(y.shape, y.dtype)
